# revision 36
# baseline (speedup 1.0000x reference)
"""Trainium2 Bass kernel for nn_GTShapelet (GIN stack + CLS-query MHA).

Self-contained: builds the Bass/Tile program, shards inputs across 8
NeuronCores (data-parallel over destination-node ranges; graphs 4c..4c+3
on core c), runs via run_bass_kernel_spmd, and reassembles the full
[32, 128] output.

Design (v2, from the 313us baseline):
  - fp8(e4m3) tables wherever the DMA/PE cost rewards it: gather tables
    (h1, h2), ct count-matrix, T1, W2/W3, selection matrices.
  - Edge slots are BAND-INTERLEAVED: within each segment, the per-chunk
    row-sorted slot blocks are interleaved across the 4 chunks in pairs,
    so gather window w only needs a prefix (~(2w+2)/17) of the AllGather
    table.  L2's gather stream overlaps L1's compute and L3's overlaps
    L2's -- the DMA engines never wait on a layer boundary.
  - AllGather pieces are split in half (256 rows) with a p-major row
    layout inside each half so the agin writes use 512B descriptors
    (full DMA speed) and the gather prefix dependency is 2048 rows.
  - Three rotating gather buffers so a buffer-WAR never couples a
    layer's first gather to the previous layer's tail.
  - DoubleRow fp8 matmuls for the ct and selection segment-sum groups
    and the node matmuls.
  - Selection matrices: first 4 segments built on DVE (overlapped with
    layer 1), rest streamed prebuilt from DRAM.
  - Gather indices stored/loaded on 32 partitions only (the gather
    ucode reads partitions 0..31).
  - Attention: K-projection folded into the query on the host; scores
    exp'd once per graph ([128,32]) to halve activation-table switches;
    the final LayerNorm uses exp(-0.5*ln(v)) so no Sqrt table load sits
    on the tail chain.
"""

import sys

if "/opt/trn_rl_repo" not in sys.path:
    sys.path.insert(0, "/opt/trn_rl_repo")

import numpy as np
import ml_dtypes

# ---- problem constants (hardcoded per spec) ----
B, N, E, D = 32, 1024, 524288, 128
H, HD = 4, 32
F2 = 2 * D                     # 256
NCORES = 8
NPC = B * N // NCORES          # 4096 nodes per core
GPC = B // NCORES              # 4 graphs per core
CHW = 128                      # dst-chunk width (nodes)
NCH = NPC // CHW               # 32 chunks per core
SEG = 4                        # chunks per segment
NSEG = NCH // SEG              # 8 segments per core
SEGN = SEG * CHW               # 512 nodes per segment
BF16 = ml_dtypes.bfloat16
FP8 = ml_dtypes.float8_e4m3

_prog_cache = {}


def _wrap16(arr):
    """slot i -> [i % 16, i // 16], replicated into partitions 16..31.

    CoreSim's gather ucode reads partitions 0..15; the deployed HW ucode
    reads 16..31 -- fill both so either path sees the indices.
    """
    n = arr.shape[0]
    out = np.zeros((32, n // 16), np.int16)
    w = arr.reshape(n // 16, 16).T.astype(np.int16)
    out[0:16] = w
    out[16:32] = w
    return out


def _mk_layout(nb, caps):
    """Class-split, band-interleaved block layout per segment.

    Slots per chunk are ordered [class-0 | class-1 | pad] where class is
    the parity of the source's p3-table row (the L3 gather fetches 256B
    row-PAIRS from the 128B-row p3 table; the class picks which half).
    caps[k] = (a, f): a pure class-0 blocks, f flex blocks straddling the
    per-core class boundary (flex blocks get TWO sel entries, one per
    class half), rest pure class-1.

    Returns per segment (ops[k4], bp, selmap, nsel):
      ops[k4]  = [(gbpos, selpos, nblk, sub)] matmul operands in band order
      bp       = (k4, j) -> gb block position
      selmap   = (k4, j) -> [(selpos, clsfilter)] for host sel building
      nsel     = sel entries in this segment
    Pure blocks of band w only need a ~(2w+2)/a prefix of the table; flex
    blocks (needing the full table) are emitted last.  The last segment is
    chunk-major (its gathers run when the table is already complete).
    """
    bands = []
    for s in range(NSEG):
        nbs = [nb[SEG * s + k4] for k4 in range(SEG)]
        afs = [caps[SEG * s + k4] for k4 in range(SEG)]
        pos = [0]
        spos = [0]
        bp = {}
        selmap = {}
        ops = [[] for _ in range(SEG)]

        def pure(k4, j, nblk, sub):
            ops[k4].append((pos[0], spos[0], nblk, sub))
            for t in range(nblk):
                bp[(k4, j + t)] = pos[0] + t
                selmap[(k4, j + t)] = [(spos[0] + t, None)]
            pos[0] += nblk
            spos[0] += nblk

        def flex(k4, j):
            ops[k4].append((pos[0], spos[0], 1, 0))
            ops[k4].append((pos[0], spos[0] + 1, 1, 1))
            bp[(k4, j)] = pos[0]
            selmap[(k4, j)] = [(spos[0], 0), (spos[0] + 1, 1)]
            pos[0] += 1
            spos[0] += 2

        def runs(k4):
            a, f = afs[k4]
            b = nbs[k4] - a - f
            return a, f, b

        if s == NSEG - 1:
            for k4 in range(SEG):
                a, f, b = runs(k4)
                j = 0
                while j < a:
                    nblk = 2 if j + 1 < a else 1
                    pure(k4, j, nblk, 0)
                    j += nblk
                j = a + f
                while j < nbs[k4]:
                    nblk = 2 if j + 1 < nbs[k4] else 1
                    pure(k4, j, nblk, 1)
                    j += nblk
                for t in range(f):
                    flex(k4, a + t)
        else:
            maxp0 = max(runs(k4)[0] // 2 for k4 in range(SEG))
            for w in range(maxp0):
                for k4 in range(SEG):
                    if w < runs(k4)[0] // 2:
                        pure(k4, 2 * w, 2, 0)
            for k4 in range(SEG):
                a = runs(k4)[0]
                if a % 2:
                    pure(k4, a - 1, 1, 0)
            maxp1 = max(runs(k4)[2] // 2 for k4 in range(SEG))
            for w in range(maxp1):
                for k4 in range(SEG):
                    a, f, b = runs(k4)
                    if w < b // 2:
                        pure(k4, a + f + 2 * w, 2, 1)
            for k4 in range(SEG):
                a, f, b = runs(k4)
                if b % 2:
                    pure(k4, nbs[k4] - 1, 1, 1)
            for k4 in range(SEG):
                a, f, b = runs(k4)
                for t in range(f):
                    flex(k4, a + t)
        bands.append((ops, bp, selmap, spos[0]))
    return bands


def _host_prep(inputs):
    node_ids = np.asarray(inputs["node_ids"]).astype(np.int64)
    src = np.asarray(inputs["src"]).astype(np.int64)
    dst = np.asarray(inputs["dst"]).astype(np.int64)
    pad_mask = np.asarray(inputs["pad_mask"])
    ew = np.asarray(inputs["edge_weight"]).astype(np.float64)
    embed = np.asarray(inputs["embed_table"]).astype(np.float64)
    W1 = np.asarray(inputs["W1"]).astype(np.float64)
    b1 = np.asarray(inputs["b1"]).astype(np.float32)
    W2 = np.asarray(inputs["W2"]).astype(np.float32)
    b2 = np.asarray(inputs["b2"]).astype(np.float32)
    W3 = np.asarray(inputs["W3"]).astype(np.float32)
    b3 = np.asarray(inputs["b3"]).astype(np.float32)
    ipw = np.asarray(inputs["in_proj_w"]).astype(np.float64)
    ipb = np.asarray(inputs["in_proj_b"]).astype(np.float64)
    ow = np.asarray(inputs["out_w"]).astype(np.float32)
    ob = np.asarray(inputs["out_b"]).astype(np.float32)
    cls = np.asarray(inputs["cls_embedding"]).astype(np.float64).reshape(D)
    ln_g = np.asarray(inputs["ln_g"]).astype(np.float32)
    ln_b = np.asarray(inputs["ln_b"]).astype(np.float32)

    assert not pad_mask.any(), "kernel compiled for all-False pad_mask"

    # ---- shared (replicated) constants ----
    T1 = (embed @ W1).astype(np.float32)                 # [1024, 256]
    t1p = T1.reshape(N // 128, 128, F2).transpose(1, 0, 2).astype(FP8)

    Wq, Wk, Wv = ipw[:, :D], ipw[:, D:2 * D], ipw[:, 2 * D:]
    bq, bk_, bv_ = ipb[:D], ipb[D:2 * D], ipb[2 * D:]
    q_cls = (cls @ Wq + bq) / np.sqrt(HD)                # [128]
    qblk = np.zeros((D, H))
    for h in range(H):
        qblk[h * HD:(h + 1) * HD, h] = q_cls[h * HD:(h + 1) * HD]
    qkf = (Wk @ qblk).astype(np.float32)                 # [128, 4]
    bkq = np.array([bk_ @ qblk[:, h] for h in range(H)])
    k_cls = cls @ Wk + bk_
    s_cls = np.array([q_cls[h * HD:(h + 1) * HD] @ k_cls[h * HD:(h + 1) * HD]
                      for h in range(H)])
    e_cls = np.exp(s_cls - bkq)                          # device scores omit bkq
    v_cls_nb = cls @ Wv                                  # bias added post-softmax
    vc4 = np.zeros((4, 128), np.float32)
    for h in range(H):
        vc4[h, h * HD:(h + 1) * HD] = v_cls_nb[h * HD:(h + 1) * HD]
    e4 = np.diag(e_cls).astype(np.float32)
    msel = np.zeros((128, 4), np.float32)
    r4 = np.zeros((4, 128), np.float32)
    for h in range(H):
        msel[h * HD:(h + 1) * HD, h] = 1.0
        r4[h, h * HD:(h + 1) * HD] = 1.0
    hsel = np.zeros((32, 4), np.float32)
    for j in range(32):
        hsel[j, j % 4] = 1.0

    w2dr = W2.reshape(2, 128, 2, 128).transpose(1, 0, 2, 3).astype(FP8)
    w3dr = W3.reshape(2, 128, 128).transpose(1, 0, 2).astype(FP8)

    # ---- edge slotting (core-uniform: program is SPMD) ----
    ew32 = ew.astype(np.float32)
    order_all = np.argsort(dst, kind='stable')
    dst_sorted = dst[order_all]
    chunk_starts = np.searchsorted(dst_sorted, np.arange(0, B * N + 1, CHW))
    cnt_all = np.diff(chunk_starts).reshape(NCORES, NCH)      # [core, chunk]
    nb_uni = np.maximum(1, -(-cnt_all.max(0) // 128)).astype(np.int64)  # [32]
    B0 = np.concatenate([[0], np.cumsum(nb_uni)]).astype(np.int64)      # [33]
    NBT = int(B0[-1])
    segslots = [int(128 * (B0[SEG * (s + 1)] - B0[SEG * s]))
                for s in range(NSEG)]
    segbase = np.concatenate([[0], np.cumsum(segslots)]).astype(np.int64)
    CAPT = int(segbase[-1])
    SEGBMAX = max(sl // 128 for sl in segslots)

    # table row permutations: node n -> agout1 row (256B h1 rows) and
    # agout2 pair-row (256B pair of 128B p3 rows).  Both are p-major
    # within a segment piece so agin writes have >=512B-contiguous
    # partitions; both are segment-major so prefix deps ramp with bands.
    nvec = np.arange(B * N, dtype=np.int64)
    c_ = nvec >> 12
    k_ = (nvec >> 7) & 31
    p_ = nvec & 127
    agrow1 = ((k_ >> 2) << 12) | (c_ << 9) | (p_ << 2) | (k_ & 3)
    agrow3p = ((k_ >> 2) << 11) | (c_ << 8) | (p_ << 1) | ((k_ & 3) >> 1)
    cls3 = k_ & 1

    # per-chunk class-0 counts across cores -> flex-block layout caps
    cls_of_edge = cls3[src]
    cnt0_all = np.zeros((NCORES, NCH), np.int64)
    for cc in range(NCORES):
        for kk_ in range(NCH):
            gk = cc * NCH + kk_
            ee = order_all[chunk_starts[gk]:chunk_starts[gk + 1]]
            cnt0_all[cc, kk_] = int((cls_of_edge[ee] == 0).sum())
    lo0 = cnt0_all.min(0)
    hi0 = cnt0_all.max(0)
    caps = []
    for kk_ in range(NCH):
        a = int(lo0[kk_] // 128)
        f = int(-(-hi0[kk_] // 128) - a)
        f = max(f, 0)
        caps.append((a, f))
    caps = tuple(caps)
    LAY = _mk_layout(tuple(int(x) for x in nb_uni), caps)
    nsel_seg = [LAY[s][3] for s in range(NSEG)]
    SB0 = np.concatenate([[0], np.cumsum(nsel_seg)]).astype(np.int64)
    NBT2 = int(SB0[-1])

    P = dict(nb=tuple(int(x) for x in nb_uni), NBT=NBT, CAPT=CAPT,
             segslots=tuple(segslots), SEGBMAX=SEGBMAX, caps=caps,
             NBT2=NBT2)
    # winrows is filled per core below and attached to P afterwards

    shared = {
        "t1p": t1p,
        "iota128": np.tile(np.arange(CHW, dtype=np.float32),
                           (128, 1)).astype(BF16),
        "w2dr": w2dr.reshape(128, 2 * 2 * 128),
        "w3dr": w3dr.reshape(128, 2 * 128),
        "b2c": b2.reshape(2, 128).T.copy(),
        "b3c": b3.reshape(128, 1).copy(),
        "wvt": Wv.astype(BF16),
        "qkf": qkf.astype(BF16),
        "i128": np.eye(128, dtype=np.float32).astype(BF16),
        "i128q": np.eye(128, dtype=np.float32).astype(FP8),
        "ones128": np.ones((128, 1), np.float32).astype(BF16),
        "hsel": hsel,
        "r4": r4,
        "msel": msel,
        "vc4": vc4.astype(BF16),
        "e4": e4.astype(BF16),
        "ecls": e_cls.astype(np.float32).reshape(4, 1),
        "bvt": bv_.astype(np.float32).reshape(128, 1),
        "ynb": (cls + ob).astype(np.float32).reshape(128, 1),
        "wo": ow.astype(BF16),
        "lngc": ln_g.reshape(128, 1).copy(),
        "lnbc": ln_b.reshape(128, 1).copy(),
        "cnts": (np.arange(9, dtype=np.int32) * 128).reshape(1, 9),
    }

    T1b = T1 + b1[None, :]                                # b1 folded into own rows
    winrows = [[0] * (-(-sl // 1024)) for sl in segslots]
    winrows3 = [[0] * (-(-sl // 1024)) for sl in segslots]
    in_maps = []
    for c in range(NCORES):
        g_idx1 = np.zeros(CAPT, np.int64)
        g_idx3 = np.zeros(CAPT, np.int64)
        selh = np.zeros((NBT2 * 128, CHW), np.float32)
        for k in range(NCH):
            seg, k4 = k // SEG, k % SEG
            gk = c * NCH + k
            a, bnd = chunk_starts[gk], chunk_starts[gk + 1]
            cnt = bnd - a
            nbk = int(nb_uni[k])
            assert cnt <= 128 * nbk
            e = order_all[a:bnd]
            # order slots [class-0 | class-1], each sorted by p3 pair-row
            # so band w's blocks only need a table prefix
            ecls = cls3[src[e]]
            r3 = agrow3p[src[e]]
            o = np.lexsort((r3, ecls))
            e = e[o]
            cnt0 = int((ecls == 0).sum())
            rows1_p = np.zeros(nbk * 128, np.int64)
            rows1_p[:cnt] = agrow1[src[e]]
            rows3_p = np.zeros(nbk * 128, np.int64)
            rows3_p[:cnt] = agrow3p[src[e]]
            dl_p = np.full(nbk * 128, -1, np.int64)
            dl_p[:cnt] = (dst[e] - (c * NPC + k * CHW)).astype(np.int64)
            ew_p = np.zeros(nbk * 128, np.float32)
            ew_p[:cnt] = ew32[e]
            bp = LAY[seg][1]
            selmap = LAY[seg][2]
            blk0 = int(B0[SEG * seg])
            sel0 = int(SB0[seg])
            for j in range(nbk):
                gb0 = (blk0 + bp[(k4, j)]) * 128
                g_idx1[gb0:gb0 + 128] = rows1_p[128 * j:128 * (j + 1)]
                g_idx3[gb0:gb0 + 128] = rows3_p[128 * j:128 * (j + 1)]
                dlj = dl_p[128 * j:128 * (j + 1)]
                for selpos, clsf in selmap[(k4, j)]:
                    sl = np.nonzero(dlj >= 0)[0]
                    if clsf == 0:
                        sl = sl[(128 * j + sl) < cnt0]
                    elif clsf == 1:
                        sl = sl[(128 * j + sl) >= cnt0]
                    sb_ = (sel0 + selpos) * 128
                    selh[sb_ + sl, dlj[sl]] = ew_p[128 * j + sl]
        eidx = order_all[chunk_starts[c * NCH]:chunk_starts[(c + 1) * NCH]]
        ids_e = node_ids[src[eidx]]
        dl_e = dst[eidx] - c * NPC
        Cf = np.bincount(dl_e * N + ids_e, weights=ew[eidx],
                         minlength=NPC * N).reshape(NPC, N).astype(np.float32)
        # ct[p, k, pr, j, d] = Cf[k*128+d, (2pr+j)*128+p]
        ct = Cf.reshape(NCH, CHW, 8, 128).transpose(3, 0, 2, 1).astype(FP8)
        nids_own = node_ids[c * NPC:(c + 1) * NPC]
        # h0own node-major [128, 32, 256], bias b1 folded in
        h0own = T1b[nids_own].reshape(NCH, CHW, F2).transpose(1, 0, 2).astype(FP8)
        for s in range(NSEG):
            sb_, se_ = int(segbase[s]), int(segbase[s + 1])
            wi = 0
            while sb_ + wi * 1024 < se_:
                w0 = sb_ + wi * 1024
                ni = min(1024, se_ - w0)
                mx = int(g_idx1[w0:w0 + ni].max())
                need = -(-(mx + 1) // 4096) * 4096
                winrows[s][wi] = max(winrows[s][wi], need)
                mx3 = int(g_idx3[w0:w0 + ni].max())
                need3 = -(-(mx3 + 1) // 2048) * 2048
                winrows3[s][wi] = max(winrows3[s][wi], need3)
                wi += 1
        m = dict(shared)
        m.update({
            "ct": np.ascontiguousarray(ct).reshape(128, NCH * 8 * CHW),
            "h0own": np.ascontiguousarray(h0own).reshape(128, NCH * F2),
            "idx12": np.concatenate([_wrap16(g_idx1), _wrap16(g_idx3)],
                                    axis=0),
            "selin": np.ascontiguousarray(
                selh.reshape(NBT2, 128, CHW).transpose(1, 0, 2)
            ).astype(FP8).reshape(128, NBT2 * CHW),
            "dstl": np.ascontiguousarray(
                np.where(selh.any(1), selh.argmax(1), 200.0)
                .reshape(NBT2, 128).T).astype(BF16),
            "eww": np.ascontiguousarray(
                selh.max(1).reshape(NBT2, 128).T).astype(BF16),
        })
        in_maps.append(m)
    P['winrows'] = tuple(tuple(w) for w in winrows)
    P['winrows3'] = tuple(tuple(w) for w in winrows3)
    return in_maps, P


def _build_program(variant, P):
    key = (variant, P['nb'], P['segslots'], P['winrows'], P['winrows3'],
           P['caps'])
    if key in _prog_cache:
        return _prog_cache[key]
    import concourse.bacc as bacc
    import concourse.tile as tile
    import concourse.mybir as mybir

    dt = mybir.dt
    AF = mybir.ActivationFunctionType
    OP = mybir.AluOpType
    DR = mybir.MatmulPerfMode.DoubleRow

    nb = P['nb']
    NBT = P['NBT']
    CAPT = P['CAPT']
    segslots = P['segslots']
    SEGBMAX = P['SEGBMAX']
    B0 = [0]
    for x in nb:
        B0.append(B0[-1] + x)
    segbase = [0]
    for sl in segslots:
        segbase.append(segbase[-1] + sl)
    winrows = P['winrows']
    winrows3 = P['winrows3']
    NBT2 = P['NBT2']
    LAY = _mk_layout(nb, P['caps'])
    SB0 = [0]
    for s in range(NSEG):
        SB0.append(SB0[-1] + LAY[s][3])

    nc = bacc.Bacc("TRN2", target_bir_lowering=False, debug=False,
                   num_devices=(1 if variant == "sim1" else NCORES))

    def din(name, shape, dtype):
        return nc.dram_tensor(name, shape, dtype, kind="ExternalInput")

    t1p = din("t1p", [128, 8, F2], dt.float8e4)
    ct = din("ct", [128, NCH * 8 * CHW], dt.float8e4)
    h0own = din("h0own", [128, NCH * F2], dt.float8e4)
    idx12 = din("idx12", [64, CAPT // 16], dt.int16)
    cnts = din("cnts", [1, 9], dt.int32)
    selin = din("selin", [128, NBT2 * CHW], dt.float8e4)
    dstl = din("dstl", [128, NBT2], dt.bfloat16)
    eww = din("eww", [128, NBT2], dt.bfloat16)
    iota128 = din("iota128", [128, CHW], dt.bfloat16)
    w2dr = din("w2dr", [128, 512], dt.float8e4)
    w3dr = din("w3dr", [128, 256], dt.float8e4)
    b2c = din("b2c", [128, 2], dt.float32)
    b3c = din("b3c", [128, 1], dt.float32)
    wvt = din("wvt", [128, 128], dt.bfloat16)
    qkf = din("qkf", [128, 4], dt.bfloat16)
    i128 = din("i128", [128, 128], dt.bfloat16)
    i128q = din("i128q", [128, 128], dt.float8e4)
    ones128 = din("ones128", [128, 1], dt.bfloat16)
    hsel = din("hsel", [32, 4], dt.float32)
    r4 = din("r4", [4, 128], dt.float32)
    msel = din("msel", [128, 4], dt.float32)
    vc4 = din("vc4", [4, 128], dt.bfloat16)
    e4 = din("e4", [4, 4], dt.bfloat16)
    ecls = din("ecls", [4, 1], dt.float32)
    bvt = din("bvt", [128, 1], dt.float32)
    ynb = din("ynb", [128, 1], dt.float32)
    wo = din("wo", [128, 128], dt.bfloat16)
    lngc = din("lngc", [128, 1], dt.float32)
    lnbc = din("lnbc", [128, 1], dt.float32)
    y_out = nc.dram_tensor("y", [GPC, D], dt.float32, kind="ExternalOutput")

    with tile.TileContext(nc) as tc:
        from concourse.library_config import mlp
        nc.gpsimd.load_library(mlp)
        with tc.tile_pool(name="const", bufs=1) as cp, \
             tc.tile_pool(name="res", bufs=1) as rp, \
             tc.tile_pool(name="work", bufs=1) as wp, \
             tc.tile_pool(name="ps", bufs=1, space="PSUM") as pp, \
             tc.tile_pool(name="dram", bufs=2, space="DRAM") as dram:

            def cload(ap, shape, dtype):
                t = cp.tile(shape, dtype, name=f"c_{ap.name}")
                nc.sync.dma_start(out=t[:], in_=ap[:])
                return t

            t1p_t = cload(t1p, [128, 8, F2], dt.float8e4)
            ctbufs = [wp.tile([128, SEG * 8 * CHW], dt.float8e4, tag=f"ctb{i}",
                              name=f"ctb{i}") for i in range(2)]
            nc.sync.dma_start(out=ctbufs[0][:], in_=ct[:, 0:SEG * 8 * CHW])
            h0own_t = cload(h0own, [128, NCH * F2], dt.float8e4)
            h0own_v = h0own_t[:].rearrange("p (k f) -> p k f", f=F2)
            idx_t = cload(idx12, [64, CAPT // 16], dt.int16)
            cnts_t = cload(cnts, [1, 9], dt.int32)
            i128_t = cload(i128, [128, 128], dt.bfloat16)
            i128q_t = cload(i128q, [128, 128], dt.float8e4)
            dstl_t = cload(dstl, [128, NBT2], dt.bfloat16)
            eww_t = cload(eww, [128, NBT2], dt.bfloat16)
            iota_t = cload(iota128, [128, CHW], dt.bfloat16)

            sel_t = rp.tile([128, NBT2, CHW], dt.float8e4, name="sel")
            hon1 = rp.tile([128, NCH, F2], dt.float8e4, name="hon1")
            hon3 = rp.tile([128, NCH, 128], dt.float8e4, name="hon3")
            p3T = rp.tile([128, NPC], dt.bfloat16, name="p3T")
            rhsT = rp.tile([128, 2, NPC], dt.float8e4, name="rhsT")
            hT3 = rp.tile([128, NPC], dt.bfloat16, name="hT3")
            vnm = rp.tile([128, NPC // 128, 128], dt.bfloat16, name="vnm")
            esc = rp.tile([128, 128], dt.bfloat16, name="esc")
            ctx_all = rp.tile([128, 4], dt.bfloat16, name="ctx_all")
            gbufs = [wp.tile([128, SEGBMAX, F2], dt.float8e4, tag=f"gb{i}",
                             name=f"gbuf{i}") for i in range(3)]

            agin1 = dram.tile([NPC, F2], dt.float8e4, tag="agin1")
            agout1 = dram.tile([B * N, F2], dt.float8e4, tag="agout1")
            agin2 = dram.tile([NPC, 128], dt.float8e4, tag="agin2")
            agout2 = dram.tile([B * N // 2, F2], dt.float8e4, tag="agout2")

            hon1_v = hon1[:]
            hon3_v = hon3[:]

            def build_sel(s, half=None):
                nbs = SB0[s + 1] - SB0[s]
                b0 = SB0[s]
                h0, h1 = 0, nbs
                if half == 0:
                    h1 = nbs // 2
                elif half == 1:
                    h0 = nbs // 2
                n = h1 - h0
                nc.vector.tensor_tensor(
                    out=sel_t[:, b0 + h0:b0 + h1, :],
                    in0=dstl_t[:, b0 + h0:b0 + h1].unsqueeze(2)
                        .broadcast_to([128, n, CHW]),
                    in1=iota_t[:].unsqueeze(1)
                        .broadcast_to([128, n, CHW]),
                    op=OP.is_equal)
                nc.vector.tensor_tensor(
                    out=sel_t[:, b0 + h0:b0 + h1, :],
                    in0=sel_t[:, b0 + h0:b0 + h1, :],
                    in1=eww_t[:, b0 + h0:b0 + h1].unsqueeze(2)
                        .broadcast_to([128, n, CHW]), op=OP.mult)

            def ag_piece1(s):
                # send this core's segment-s h1 rows (p-major: each
                # partition writes 4 rows = 1KB contiguous), recv all
                agin_v = agin1.rearrange("(g p t) f -> p g t f",
                                         t=SEG, p=CHW)
                nc.sync.dma_start(
                    out=agin_v[:, s, :, :],
                    in_=hon1_v[:, SEG * s:SEG * (s + 1), :])
                r0 = s * NCORES * SEGN
                if variant == "sim1":
                    nc.sync.dma_start(
                        out=agout1[r0:r0 + NCORES * SEGN, :].rearrange(
                            "(c n) f -> c n f", c=NCORES),
                        in_=agin1[s * SEGN:(s + 1) * SEGN, :].unsqueeze(0)
                            .broadcast_to([NCORES, SEGN, F2]))
                else:
                    nc.gpsimd.collective_compute(
                        "AllGather", mybir.AluOpType.bypass,
                        replica_groups=[list(range(NCORES))],
                        ins=[agin1[s * SEGN:(s + 1) * SEGN, :].opt()],
                        outs=[agout1[r0:r0 + NCORES * SEGN, :].opt()])

            def ag_piece2(s):
                # p3 rows are 128B; the table is pair-major [16384, 256]
                agin_v = agin2.rearrange("(g p t) f -> p g t f",
                                         t=SEG, p=CHW)
                nc.sync.dma_start(
                    out=agin_v[:, s, :, :],
                    in_=hon3_v[:, SEG * s:SEG * (s + 1), :])
                npair = SEGN // 2
                r0 = s * NCORES * npair
                if variant == "sim1":
                    nc.sync.dma_start(
                        out=agout2[r0:r0 + NCORES * npair, :].rearrange(
                            "(c n) f -> c n f", c=NCORES),
                        in_=agin2[s * SEGN:(s + 1) * SEGN, :].rearrange(
                            "(n two) f -> n (two f)", two=2).unsqueeze(0)
                            .broadcast_to([NCORES, npair, F2]))
                else:
                    nc.gpsimd.collective_compute(
                        "AllGather", mybir.AluOpType.bypass,
                        replica_groups=[list(range(NCORES))],
                        ins=[agin2[s * SEGN:(s + 1) * SEGN, :].opt()],
                        outs=[agout2[r0:r0 + NCORES * npair, :].opt()])

            # selection matrices for segments 5-7 stream from DRAM
            # (issued first so they land early in the DMA stream); 0-4
            # build on the DVE interleaved with L1/L2
            for s_ in (5, 6, 7):
                nbs_ = SB0[s_ + 1] - SB0[s_]
                b0_ = SB0[s_]
                nc.sync.dma_start(
                    out=sel_t[:, b0_:b0_ + nbs_, :],
                    in_=selin[:, b0_ * CHW:(b0_ + nbs_) * CHW].rearrange(
                        "p (b d) -> p b d", d=CHW))

            # ---------------- layer 1 (ct x T1) ----------------
            for s in range(NSEG):
                if s + 1 < NSEG:
                    nc.sync.dma_start(
                        out=ctbufs[(s + 1) % 2][:],
                        in_=ct[:, (s + 1) * SEG * 8 * CHW:(s + 2) * SEG * 8 * CHW])
                ctb_v = ctbufs[s % 2][:].rearrange(
                    "p (kk pr j d) -> p kk pr j d", pr=4, j=2, d=CHW)
                for kk in range(SEG):
                    k = SEG * s + kk
                    ps = pp.tile([CHW, F2], dt.float32, tag="big", bufs=2)
                    for pr in range(4):
                        nc.tensor.matmul(
                            out=ps[:], lhsT=ctb_v[:, kk, pr, :, :],
                            rhs=t1p_t[:, 2 * pr:2 * pr + 2, :],
                            perf_mode=DR, start=(pr == 0), stop=False,
                            skip_group_check=True)
                    nc.tensor.matmul(
                        out=ps[:], lhsT=i128q_t[:], rhs=h0own_v[:, k, :],
                        start=False, stop=True, skip_group_check=True)
                    nc.scalar.activation(hon1_v[:, k, :], ps[:], AF.Gelu)
                # selection matrices for this segment (used by L2+L3):
                # segments 0-3 build on the DVE here; 4-7 are interleaved
                # into the L2 loop so the DVE queue stays in need-order
                if s < 4:
                    build_sel(s)
                ag_piece1(s)

            # small constants: issued after L1's streams so they don't
            # block the prologue on HWDGE; they land in the L1->L2 gap
            w2_t = cload(w2dr, [128, 512], dt.float8e4)
            w2_v = w2_t[:].rearrange("p (t jo o) -> p t jo o", t=2, jo=2)
            w3_t = cload(w3dr, [128, 256], dt.float8e4)
            w3_v = w3_t[:].rearrange("p (t o) -> p t o", t=2)
            b2c_t = cload(b2c, [128, 2], dt.float32)
            b3c_t = cload(b3c, [128, 1], dt.float32)
            wvt_t = cload(wvt, [128, 128], dt.bfloat16)
            qkf_t = cload(qkf, [128, 4], dt.bfloat16)
            ones_t = cload(ones128, [128, 1], dt.bfloat16)
            hsel_t = cload(hsel, [32, 4], dt.float32)
            r4_t = cload(r4, [4, 128], dt.float32)
            msel_t = cload(msel, [128, 4], dt.float32)
            vc4_t = cload(vc4, [4, 128], dt.bfloat16)
            e4_t = cload(e4, [4, 4], dt.bfloat16)
            ecls_t = cload(ecls, [4, 1], dt.float32)
            bvt_t = cload(bvt, [128, 1], dt.float32)
            ynb_t = cload(ynb, [128, 1], dt.float32)
            wo_t = cload(wo, [128, 128], dt.bfloat16)
            lngc_t = cload(lngc, [128, 1], dt.float32)
            lnbc_t = cload(lnbc, [128, 1], dt.float32)

            # ---------------- layers 2 and 3 ----------------
            cregs = {}
            psc32 = None
            for layer in (2, 3):
                for s in range(NSEG):
                    gb = gbufs[((layer - 2) * NSEG + s) % 3]
                    ss_ = segslots[s]
                    bops = LAY[s][0]
                    b0 = SB0[s]
                    ci = 0
                    while ci * 1024 < ss_:
                        w0 = ci * 1024
                        ni = min(1024, ss_ - w0)
                        nbw = ni // 128
                        if nbw not in cregs:
                            cregs[nbw] = nc.gpsimd.value_load(
                                cnts_t[0:1, nbw:nbw + 1])
                        i0 = (segbase[s] + w0) // 16
                        if layer == 2:
                            nc.gpsimd.dma_gather(
                                gb[:, w0 // 128:w0 // 128 + nbw, :],
                                agout1[0:winrows[s][ci], :],
                                idx_t[0:32, i0:i0 + ni // 16],
                                ni, cregs[nbw], F2)
                        else:
                            nc.gpsimd.dma_gather(
                                gb[:, w0 // 128:w0 // 128 + nbw, :],
                                agout2[0:winrows3[s][ci], :],
                                idx_t[32:64, i0:i0 + ni // 16],
                                ni, cregs[nbw], F2)
                        ci += 1
                    if layer == 2:
                        # normal orientation: out = [dst, 256] node-major
                        for kk in range(SEG):
                            k = SEG * s + kk
                            ops = bops[kk]
                            ps = pp.tile([CHW, F2], dt.float32, tag="big",
                                         bufs=2)
                            # seed with the own rows (h + msg): no DVE add
                            nc.tensor.matmul(
                                out=ps[:], lhsT=i128q_t[:], rhs=hon1_v[:, k, :],
                                start=True, stop=False, skip_group_check=True)
                            for oi, (pos, selpos, nblk, sub) in enumerate(ops):
                                sp = (oi == len(ops) - 1)
                                if nblk == 2:
                                    nc.tensor.matmul(
                                        out=ps[:],
                                        lhsT=sel_t[:, b0 + selpos:
                                                   b0 + selpos + 2, :],
                                        rhs=gb[:, pos:pos + 2, :],
                                        perf_mode=DR, start=False, stop=sp,
                                        skip_group_check=True)
                                else:
                                    nc.tensor.matmul(
                                        out=ps[:],
                                        lhsT=sel_t[:, b0 + selpos, :],
                                        rhs=gb[:, pos, :],
                                        start=False, stop=sp,
                                        skip_group_check=True)
                            msb = wp.tile([CHW, F2], dt.bfloat16, tag="msb",
                                          bufs=3)
                            nc.scalar.activation(msb[:], ps[:], AF.Copy)
                            for jj in range(2):
                                tp = pp.tile([128, 128], dt.bfloat16,
                                             tag="tp", bufs=2)
                                nc.tensor.transpose(
                                    tp[:], msb[:, jj * 128:(jj + 1) * 128],
                                    i128_t[:])
                                nc.scalar.activation(
                                    rhsT[:, jj, k * CHW:(k + 1) * CHW], tp[:],
                                    AF.Copy)
                        # node matmuls: h2 halves (fp8) then p3 = h2 @ W3,
                        # kept feature-major (p3T) for the L3 seed and
                        # node-major (hon3) for the AllGather piece
                        for kk in range(SEG):
                            k = SEG * s + kk
                            hf2 = wp.tile([128, 2, 128], dt.float8e4,
                                          tag="hf", bufs=3)
                            for jo in range(2):
                                psz = pp.tile([128, 128], dt.float32,
                                              tag="tp", bufs=2)
                                nc.tensor.matmul(
                                    out=psz[:], lhsT=w2_v[:, :, jo, :],
                                    rhs=rhsT[:, :, k * CHW:(k + 1) * CHW],
                                    perf_mode=DR, start=True, stop=True,
                                    skip_group_check=True)
                                nc.scalar.activation(hf2[:, jo, :], psz[:],
                                                     AF.Gelu,
                                                     bias=b2c_t[:, jo:jo + 1])
                            psp = pp.tile([128, 128], dt.float32,
                                          tag="tp", bufs=2)
                            nc.tensor.matmul(
                                out=psp[:], lhsT=w3_v[:, :, :], rhs=hf2[:],
                                perf_mode=DR, start=True, stop=True,
                                skip_group_check=True)
                            nc.scalar.activation(
                                p3T[:, k * CHW:(k + 1) * CHW], psp[:], AF.Copy)
                            tp3 = pp.tile([128, 128], dt.bfloat16,
                                          tag="tp", bufs=2)
                            nc.tensor.transpose(
                                tp3[:], p3T[:, k * CHW:(k + 1) * CHW],
                                i128_t[:])
                            nc.scalar.activation(hon3_v[:, k, :], tp3[:],
                                                 AF.Copy)
                        ag_piece2(s)
                        # sel build for segment 4 slots into the DVE
                        # queue here, after the segment's own DVE work
                        if s == 0:
                            build_sel(4, half=0)
                        elif s == 1:
                            build_sel(4, half=1)
                    else:
                        if s % 2 == 0:
                            psc32 = pp.tile([128, 32], dt.float32, tag="psc",
                                            bufs=1)
                        # swapped orientation: out = [feat, dst] directly
                        # (lhsT = gathered p3 pair-halves, rhs = sel), so no
                        # transposes, W3 matmul, or DVE add in layer 3
                        for kk in range(SEG):
                            k = SEG * s + kk
                            ops = bops[kk]
                            ps = pp.tile([CHW, F2], dt.float32, tag="big",
                                         bufs=2)
                            nc.tensor.matmul(
                                out=ps[:, 0:128], lhsT=i128_t[:],
                                rhs=p3T[:, k * CHW:(k + 1) * CHW],
                                start=True, stop=False, skip_group_check=True)
                            for oi, (pos, selpos, nblk, sub) in enumerate(ops):
                                sp = (oi == len(ops) - 1)
                                if nblk == 2:
                                    nc.tensor.matmul(
                                        out=ps[:, 0:128],
                                        lhsT=gb[:, pos:pos + 2,
                                                sub * 128:(sub + 1) * 128],
                                        rhs=sel_t[:, b0 + selpos:
                                                  b0 + selpos + 2, :],
                                        perf_mode=DR, start=False, stop=sp,
                                        skip_group_check=True)
                                else:
                                    nc.tensor.matmul(
                                        out=ps[:, 0:128],
                                        lhsT=gb[:, pos,
                                                sub * 128:(sub + 1) * 128],
                                        rhs=sel_t[:, b0 + selpos, :],
                                        start=False, stop=sp,
                                        skip_group_check=True)
                            nc.scalar.activation(
                                hT3[:, k * CHW:(k + 1) * CHW],
                                ps[:, 0:128], AF.Gelu, bias=b3c_t[:, 0:1])
                            # attention scores for this chunk: col block
                            # (s%2)*16 + kk*4 of the graph's [128,32] psum
                            cb = (s % 2) * 16 + kk * 4
                            nc.tensor.matmul(
                                out=psc32[:, cb:cb + 4],
                                lhsT=hT3[:, k * 128:(k + 1) * 128], rhs=qkf_t[:],
                                start=True, stop=True)
                            psv = pp.tile([128, 128], dt.float32, tag="att",
                                          bufs=2)
                            nc.tensor.matmul(
                                out=psv[:], lhsT=hT3[:, k * 128:(k + 1) * 128],
                                rhs=wvt_t[:], start=True, stop=True)
                            nc.scalar.activation(vnm[:, k, :], psv[:], AF.Copy)
                        if s % 2 == 1:
                            g = s // 2
                            # one Exp per graph keeps Gelu<->Exp table
                            # switches off the per-segment path
                            nc.scalar.activation(esc[:, g * 32:(g + 1) * 32],
                                                 psc32[:], AF.Exp)
                            # numerator accumulation first: PE is in-order,
                            # so psg runs while the psE->rbc chain ping-pongs
                            # between PE and DVE
                            psg = pp.tile([128, 4], dt.float32, tag="psg", bufs=1)
                            for t in range(8):
                                nc.tensor.matmul(
                                    out=psg[:], lhsT=vnm[:, 8 * g + t, :],
                                    rhs=esc[:, (8 * g + t) * 4:
                                            (8 * g + t + 1) * 4],
                                    start=(t == 0), stop=False,
                                    skip_group_check=True)
                            nc.tensor.matmul(out=psg[:], lhsT=vc4_t[:],
                                             rhs=e4_t[:], start=False, stop=True,
                                             skip_group_check=True)
                            psE = pp.tile([32, 1], dt.float32, tag="att", bufs=2)
                            nc.tensor.matmul(out=psE[:],
                                             lhsT=esc[:, g * 32:(g + 1) * 32],
                                             rhs=ones_t[:], start=True, stop=True)
                            s32 = wp.tile([32, 1], dt.float32, tag="s32", bufs=2)
                            nc.vector.tensor_copy(out=s32[:], in_=psE[:])
                            ps4 = pp.tile([4, 1], dt.float32, tag="att", bufs=2)
                            nc.tensor.matmul(out=ps4[:], lhsT=hsel_t[:],
                                             rhs=s32[:], start=True, stop=True)
                            sums4 = wp.tile([4, 1], dt.float32, tag="sums4",
                                            bufs=2)
                            nc.vector.tensor_tensor(out=sums4[:], in0=ps4[:],
                                                    in1=ecls_t[:], op=OP.add)
                            rr4 = wp.tile([4, 1], dt.float32, tag="rr4", bufs=2)
                            nc.vector.reciprocal(rr4[:], sums4[:])
                            psr = pp.tile([128, 1], dt.float32, tag="att", bufs=2)
                            nc.tensor.matmul(out=psr[:], lhsT=r4_t[:],
                                             rhs=rr4[:], start=True, stop=True)
                            rbc = wp.tile([128, 1], dt.float32, tag="rbc", bufs=2)
                            nc.vector.tensor_copy(out=rbc[:], in_=psr[:])
                            tmp4 = wp.tile([128, 4], dt.float32, tag="tmp4",
                                           bufs=2)
                            nc.vector.tensor_tensor(out=tmp4[:], in0=psg[:],
                                                    in1=msel_t[:], op=OP.mult)
                            ctxv = wp.tile([128, 1], dt.float32, tag="ctxv",
                                           bufs=2)
                            nc.vector.reduce_sum(out=ctxv[:], in_=tmp4[:],
                                                 axis=mybir.AxisListType.X)
                            nc.vector.tensor_scalar(
                                out=ctx_all[:, g:g + 1], in0=ctxv[:],
                                scalar1=rbc[:], scalar2=bvt_t[:],
                                op0=OP.mult, op1=OP.add)

            # ---------------- output projection + LayerNorm ----------------
            psao = pp.tile([128, 4], dt.float32, tag="att", bufs=2)
            nc.tensor.matmul(out=psao[:], lhsT=wo_t[:], rhs=ctx_all[:],
                             start=True, stop=True)
            ysb = wp.tile([128, 4], dt.float32, tag="ysb")
            nc.vector.tensor_scalar(out=ysb[:], in0=psao[:],
                                    scalar1=ynb_t[:], scalar2=None, op0=OP.add)
            # per-graph LayerNorm on the GPSIMD layernorm ucode: one token of
            # 128 partitions x 1 free elem, so no activation-table loads or
            # DVE reduction chain sit on the tail
            ynF = wp.tile([128, 4], dt.float32, tag="ynF")
            for g in range(GPC):
                nc.gpsimd.layernorm(
                    ynF[:, g:g + 1], ysb[:, g:g + 1],
                    gamma_ap=lngc_t[:], beta_ap=lnbc_t[:],
                    eps=1e-5, subtract_mean=True, n_tokens=1)
            nc.sync.dma_start(out=y_out[:].rearrange("g d -> d g"), in_=ynF[:])

    nc.compile()
    _prog_cache[key] = nc
    return nc


def kernel(**inputs):
    from concourse.bass_utils import run_bass_kernel_spmd
    in_maps, P = _host_prep(inputs)
    nc = _build_program("hw", P)
    res = run_bass_kernel_spmd(nc, in_maps, core_ids=list(range(NCORES)))
    y = np.concatenate([res.results[c]["y"] for c in range(NCORES)], axis=0)
    return np.ascontiguousarray(y.astype(np.float32))
